# revision 3
# baseline (speedup 1.0000x reference)
"""MoE (top-2 routing, E=8 experts) Trainium2 kernel.

Strategy (quad expert-groups x F-sliced expert-parallel, 8 NeuronCores):
  - Host: gate in fp64, top-2 + softmax exactly reproducing reference routing.
  - Experts are ranked by routed-token count and paired into 4 "slots"
    (rank 2s with rank 2s+1); one expert of each slot goes to quad A
    (cores 0-3), the other to quad B (cores 4-7). Each core of a quad
    holds a distinct quarter of the F dimension (1024 rows of w1 /
    columns of w2) for all 4 of its quad's experts, and processes ALL
    routed tokens of those experts on its F-quarter. Per-core matmul
    work is therefore ~sum(slot capacities) * D * (F/4) * 2 regardless
    of routing imbalance -- near-perfect load balance (~110us of PE
    stream vs ~116us for 1-expert-per-core).
  - Device (per core, per slot): h.T = gelu(w1q @ x.T) [1024 x C],
    y.T += w2q @ h.T (partial over the F-quarter), evicted PSUM->SBUF
    as bf16 by the DVE and DMA'd out. Stage-1 of slot s+1 is interleaved
    with stage-2 of slot s at matmul-group granularity so the PE never
    waits on the gelu eviction.
  - Host: combine -- sum the 4 partial y's per quad (fp32), scale by the
    routing coefficient, scatter-add the two expert contributions.
"""

import sys
from itertools import zip_longest

import ml_dtypes
import numpy as np

if "/opt/trn_rl_repo" not in sys.path:
    sys.path.insert(0, "/opt/trn_rl_repo")

import concourse.tile as tile
from concourse import bacc, mybir
from concourse.bass_utils import run_bass_kernel_spmd

BF16 = ml_dtypes.bfloat16

E, D, F, NTOK = 8, 1024, 4096, 2048
P = 128
KD = D // P  # 8: stage-1 contraction tiles; also output d-tiles
NQ = 4  # experts per quad == F-quarters per quad
FQ = F // NQ  # 1024 rows of F per core
KF = FQ // P  # 8: f-tiles per slot; also stage-2 contraction tiles
PSUM_FREE = 512  # one PSUM bank of fp32

_NC_CACHE: dict = {}


def _plan(cnt_max: int):
    """Slot capacity C = nch * L, L <= 512, both multiples of 16."""
    C = max(16, -(-cnt_max // 16) * 16)
    nch = -(-C // 512)
    L = -(-C // (nch * 16)) * 16
    return nch * L, L, nch


def _build_nc(Cs, Ls):
    nchs = [c // l for c, l in zip(Cs, Ls)]
    Cmax = max(Cs)
    nc = bacc.Bacc(None)
    w1_in = [
        nc.declare_dram_parameter(
            f"w1_{s}", [KF, P, KD, P], mybir.dt.bfloat16, isOutput=False
        )
        for s in range(NQ)
    ]
    w2_in = [
        nc.declare_dram_parameter(
            f"w2_{s}", [P, KD, KF, P], mybir.dt.bfloat16, isOutput=False
        )
        for s in range(NQ)
    ]
    x_in = [
        nc.declare_dram_parameter(
            f"x_{s}", [nchs[s], P, KD, Ls[s]], mybir.dt.bfloat16, isOutput=False
        )
        for s in range(NQ)
    ]
    y_out = [
        nc.declare_dram_parameter(
            f"y_{s}", [KD, P, Cs[s]], mybir.dt.bfloat16, isOutput=True
        )
        for s in range(NQ)
    ]

    with tile.TileContext(nc) as tc:
        with (
            tc.tile_pool(name="wpool", bufs=1) as wpool,
            tc.tile_pool(name="apool", bufs=1) as apool,
            tc.tile_pool(name="hpool", bufs=1) as hpool,
            tc.tile_pool(name="ypool", bufs=1) as ypool,
            tc.tile_pool(name="ps1", bufs=4, space="PSUM") as ps1,
            tc.tile_pool(name="ps2", bufs=3, space="PSUM") as ps2,
            tc.tile_pool(name="psw", bufs=1, space="PSUM") as psw,
        ):
            # ---- PE warm-up: dependency-free matmuls during the DMA head so
            # the HAM clock gate ramps toward 8/8 before the real stream.
            warm = wpool.tile([P, P], mybir.dt.bfloat16, name="warm")
            nc.vector.memset(warm, 0.0)
            N_WARM = 24
            ps_w = psw.tile([P, 64], mybir.dt.float32, name="ps_w")
            for i in range(N_WARM):
                nc.tensor.matmul(
                    ps_w, warm, warm[:, :64], start=(i == 0), stop=(i == N_WARM - 1)
                )

            # ---- input loads, in consumption order on one queue.
            w1_sb = [[None] * KF for _ in range(NQ)]
            w2_sb = [None] * NQ
            x_sb = [[None] * nchs[s] for s in range(NQ)]

            def load_w1(s, f):
                t = wpool.tile([P, KD, P], mybir.dt.bfloat16, name=f"w1_{s}_{f}")
                nc.sync.dma_start(t, w1_in[s][f])
                w1_sb[s][f] = t

            def load_x(s, c):
                t = apool.tile([P, KD, Ls[s]], mybir.dt.bfloat16, name=f"x_{s}_{c}")
                nc.sync.dma_start(t, x_in[s][c])
                x_sb[s][c] = t

            def load_w2(s):
                t = wpool.tile([P, KD, KF, P], mybir.dt.bfloat16, name=f"w2_{s}")
                nc.sync.dma_start(t, w2_in[s][:])
                w2_sb[s] = t

            load_w1(0, 0)
            load_x(0, 0)
            for f in range(1, KF):
                load_w1(0, f)
            for c in range(1, nchs[0]):
                load_x(0, c)
            for f in range(KF):
                load_w1(1, f)
            for c in range(nchs[1]):
                load_x(1, c)
            load_w2(0)
            for f in range(KF):
                load_w1(2, f)
            for c in range(nchs[2]):
                load_x(2, c)
            load_w2(1)
            for f in range(KF):
                load_w1(3, f)
            for c in range(nchs[3]):
                load_x(3, c)
            load_w2(2)
            load_w2(3)

            h_sb: dict = {}
            y_sb: dict = {}

            # stage 1 group: h.T[f-tile, chunk] = gelu(w1q[f] @ x.T[chunk])
            def g1(s, f, c):
                L = Ls[s]
                if (s, f) not in h_sb:
                    h_sb[(s, f)] = hpool.tile(
                        [P, Cmax],
                        mybir.dt.bfloat16,
                        name=f"h_{s}_{f}",
                        tag="h",
                        bufs=2 * KF,
                    )
                ps = ps1.tile([P, PSUM_FREE], mybir.dt.float32, name="ps1t", tag="ps1t")
                for k in range(KD):
                    nc.tensor.matmul(
                        ps[:, :L],
                        w1_sb[s][f][:, k],
                        x_sb[s][c][:, k],
                        start=(k == 0),
                        stop=(k == KD - 1),
                    )
                nc.scalar.activation(
                    out=h_sb[(s, f)][:, c * L : (c + 1) * L],
                    in_=ps[:, :L],
                    func=mybir.ActivationFunctionType.Gelu,
                )

            # stage 2 group: y.T[d-tile, chunk] = w2q[d] @ h.T[chunk] (partial)
            def g2(s, d, c):
                L = Ls[s]
                ps = ps2.tile([P, PSUM_FREE], mybir.dt.float32, name="ps2t", tag="ps2t")
                for k2 in range(KF):
                    nc.tensor.matmul(
                        ps[:, :L],
                        w2_sb[s][:, d, k2],
                        h_sb[(s, k2)][:, c * L : (c + 1) * L],
                        start=(k2 == 0),
                        stop=(k2 == KF - 1),
                    )
                if c == 0:
                    y_sb[(s, d)] = ypool.tile(
                        [P, Cmax], mybir.dt.bfloat16, name=f"y_{s}_{d}", tag="y", bufs=8
                    )
                nc.vector.tensor_copy(y_sb[(s, d)][:, c * L : (c + 1) * L], ps[:, :L])
                if c == nchs[s] - 1:
                    nc.sync.dma_start(y_out[s][d], y_sb[(s, d)][:, : Cs[s]])

            # ---- interleave stage-1 of slot s with stage-2 of slot s-1 so the
            # PE never stalls on the gelu eviction between stages.
            for phase in range(NQ + 1):
                g1s = (
                    [(phase, f, c) for f in range(KF) for c in range(nchs[phase])]
                    if phase < NQ
                    else []
                )
                g2s = (
                    [
                        (phase - 1, d, c)
                        for d in range(KD)
                        for c in range(nchs[phase - 1])
                    ]
                    if phase >= 1
                    else []
                )
                for a, b in zip_longest(g1s, g2s):
                    if a is not None:
                        g1(*a)
                    if b is not None:
                        g2(*b)
    nc.finalize()
    return nc


def _route(x: np.ndarray, gate_w: np.ndarray):
    """fp64 gating; matches reference fp32 routing (selection gaps >> fp32 eps)."""
    logits = x.astype(np.float64) @ gate_w.astype(np.float64).T  # [N, E]
    top2 = np.argsort(-logits, axis=1, kind="stable")[:, :2]  # [N, 2]
    v = np.take_along_axis(logits, top2, axis=1)
    v = v - v.max(axis=1, keepdims=True)
    ew = np.exp(v)
    w = ew / ew.sum(axis=1, keepdims=True)  # [N, 2]
    return top2, w.astype(np.float32)


def _run(inputs: dict, trace: bool = False, trace_cores=None):
    x = np.asarray(inputs["x"], dtype=np.float32)
    gate_w = np.asarray(inputs["gate_w"], dtype=np.float32)
    w1 = np.asarray(inputs["w1"], dtype=np.float32)
    w2 = np.asarray(inputs["w2"], dtype=np.float32)
    n = x.shape[0]

    top2, wsm = _route(x, gate_w)

    idx_list, coef_list = [], []
    for e in range(E):
        mask = top2 == e  # [N, 2]
        sel = mask.any(axis=1)
        idx = np.nonzero(sel)[0]
        we = np.where(mask[idx, 0], wsm[idx, 0], wsm[idx, 1])
        idx_list.append(idx)
        coef_list.append(we.astype(np.float32))
    counts = np.array([len(i) for i in idx_list])

    # slot s holds the 2s-th and (2s+1)-th busiest experts, snaked across
    # the two quads to balance quad sums.
    order = np.argsort(-counts, kind="stable")
    quad = [[0] * NQ for _ in range(2)]
    for s in range(NQ):
        a, b = int(order[2 * s]), int(order[2 * s + 1])
        if s % 2 == 1:
            a, b = b, a
        quad[0][s], quad[1][s] = a, b

    Cs, Ls, nchs = [], [], []
    for s in range(NQ):
        C, L, nch = _plan(int(max(counts[quad[0][s]], counts[quad[1][s]])))
        Cs.append(C)
        Ls.append(L)
        nchs.append(nch)

    key = (tuple(Cs), tuple(Ls))
    if key not in _NC_CACHE:
        _NC_CACHE[key] = _build_nc(Cs, Ls)
    nc = _NC_CACHE[key]

    in_maps = []
    for q in range(2):
        # packs shared by the 4 cores of a quad, except the F-slice
        xpacks = {}
        for s in range(NQ):
            e = quad[q][s]
            idx = idx_list[e]
            cnt = len(idx)
            C, L, nch = Cs[s], Ls[s], nchs[s]
            xT = np.zeros((D, C), dtype=BF16)
            xT[:, :cnt] = x[idx].T.astype(BF16)
            # [D, C] -> pack [c, p, k, t]
            xpacks[s] = np.ascontiguousarray(
                xT.reshape(KD, P, nch, L).transpose(2, 1, 0, 3)
            )
        for i in range(NQ):
            fsl = slice(i * FQ, (i + 1) * FQ)
            feed = {}
            for s in range(NQ):
                e = quad[q][s]
                feed[f"x_{s}"] = xpacks[s]
                # w1[e][fsl] is [FQ, D]: [f, c, k, p] -> pack [f, p, k, c]
                feed[f"w1_{s}"] = np.ascontiguousarray(
                    w1[e][fsl].astype(BF16).reshape(KF, P, KD, P).transpose(0, 3, 2, 1)
                )
                # w2[e][:, fsl] is [D, FQ]: [d, c, k2, p] -> pack [p, d, k2, c]
                feed[f"w2_{s}"] = np.ascontiguousarray(
                    w2[e][:, fsl]
                    .astype(BF16)
                    .reshape(KD, P, KF, P)
                    .transpose(3, 0, 2, 1)
                )
            in_maps.append(feed)

    res = run_bass_kernel_spmd(
        nc,
        in_maps,
        list(range(E)),
        trace=trace,
        trace_cores=trace_cores,
    )

    out = np.zeros((n, D), dtype=np.float32)
    for q in range(2):
        for s in range(NQ):
            e = quad[q][s]
            idx = idx_list[e]
            cnt = len(idx)
            ysum = np.zeros((D, Cs[s]), dtype=np.float32)
            for i in range(NQ):
                ysum += np.asarray(
                    res.results[4 * q + i][f"y_{s}"], dtype=np.float32
                ).reshape(D, Cs[s])
            out[idx] += (ysum[:, :cnt] * coef_list[e][None, :]).T
    return out, res


def kernel(**inputs) -> np.ndarray:
    out, _ = _run(inputs, trace=False)
    return out


if __name__ == "__main__":
    rng = np.random.default_rng(0)
    fake = {
        "x": rng.standard_normal((NTOK, D), dtype=np.float32),
        "gate_w": (rng.standard_normal((E, D)) * 0.02).astype(np.float32),
        "w1": (rng.standard_normal((E, F, D)) * 0.02).astype(np.float32),
        "w2": (rng.standard_normal((E, D, F)) * 0.02).astype(np.float32),
    }
    out = kernel(**fake)
    print("ok", out.shape, out.dtype, np.abs(out).max())


# revision 6
# speedup vs baseline: 1.0571x; 1.0571x over previous
"""MoE (top-2 routing, E=8 experts) Trainium2 kernel.

Strategy (quad expert-groups x F-sliced expert-parallel, 8 NeuronCores):
  - Host: gate in fp64, top-2 + softmax exactly reproducing reference routing.
  - Experts are ranked by routed-token count and paired into 4 "slots"
    (rank 2s with rank 2s+1); one expert of each slot goes to quad A
    (cores 0-3), the other to quad B (cores 4-7). Each core of a quad
    holds a distinct quarter of the F dimension (1024 rows of w1 /
    columns of w2) for all 4 of its quad's experts, and processes ALL
    routed tokens of those experts on its F-quarter. Per-core matmul
    work is therefore ~sum(slot capacities) * D * (F/4) * 2 regardless
    of routing imbalance -- near-perfect load balance (~110us of PE
    stream vs ~116us for 1-expert-per-core).
  - Device (per core, per slot): h.T = gelu(w1q @ x.T) [1024 x C],
    y.T += w2q @ h.T (partial over the F-quarter), evicted PSUM->SBUF
    as bf16 by the DVE and DMA'd out. Stage-1 of slot s+1 is interleaved
    with stage-2 of slot s at matmul-group granularity so the PE never
    waits on the gelu eviction.
  - Host: combine -- sum the 4 partial y's per quad (fp32), scale by the
    routing coefficient, scatter-add the two expert contributions.
"""

import sys
from itertools import zip_longest

import ml_dtypes
import numpy as np

if "/opt/trn_rl_repo" not in sys.path:
    sys.path.insert(0, "/opt/trn_rl_repo")

import concourse.tile as tile
from concourse import bacc, mybir
from concourse.bass_utils import run_bass_kernel_spmd

BF16 = ml_dtypes.bfloat16

E, D, F, NTOK = 8, 1024, 4096, 2048
P = 128
KD = D // P  # 8: stage-1 contraction tiles; also output d-tiles
NQ = 4  # experts per quad == F-quarters per quad
FQ = F // NQ  # 1024 rows of F per core
KF = FQ // P  # 8: f-tiles per slot; also stage-2 contraction tiles
PSUM_FREE = 512  # one PSUM bank of fp32

_NC_CACHE: dict = {}


def _plan(cnt_max: int):
    """Slot capacity C = nch * L, L <= 512, both multiples of 16."""
    C = max(16, -(-cnt_max // 16) * 16)
    nch = -(-C // 512)
    L = -(-C // (nch * 16)) * 16
    return nch * L, L, nch


def _build_nc(Cs, Ls):
    nchs = [c // l for c, l in zip(Cs, Ls)]
    Cmax = max(Cs)
    nc = bacc.Bacc(None)
    w1_in = [
        nc.declare_dram_parameter(
            f"w1_{s}", [KF, P, KD, P], mybir.dt.bfloat16, isOutput=False
        )
        for s in range(NQ)
    ]
    w2_in = [
        nc.declare_dram_parameter(
            f"w2_{s}", [P, KD, KF, P], mybir.dt.bfloat16, isOutput=False
        )
        for s in range(NQ)
    ]
    x_in = [
        nc.declare_dram_parameter(
            f"x_{s}", [nchs[s], P, KD, Ls[s]], mybir.dt.bfloat16, isOutput=False
        )
        for s in range(NQ)
    ]
    y_out = [
        nc.declare_dram_parameter(
            f"y_{s}", [KD, P, Cs[s]], mybir.dt.bfloat16, isOutput=True
        )
        for s in range(NQ)
    ]

    with tile.TileContext(nc) as tc:
        with (
            tc.tile_pool(name="wpool", bufs=1) as wpool,
            tc.tile_pool(name="apool", bufs=1) as apool,
            tc.tile_pool(name="hpool", bufs=1) as hpool,
            tc.tile_pool(name="ypool", bufs=1) as ypool,
            tc.tile_pool(name="ps1", bufs=4, space="PSUM") as ps1,
            tc.tile_pool(name="ps2", bufs=3, space="PSUM") as ps2,
            tc.tile_pool(name="psw", bufs=1, space="PSUM") as psw,
        ):
            # ---- PE warm-up: dependency-free matmuls during the DMA head so
            # the HAM clock gate is at 8/8 by the time the real stream starts
            # (the first x/w1 DMAs land ~7us after the PE frees up).
            warm = wpool.tile([P, P], mybir.dt.bfloat16, name="warm")
            nc.vector.memset(warm, 0.0)
            N_WARM = 80
            ps_w = psw.tile([P, 64], mybir.dt.float32, name="ps_w")
            for i in range(N_WARM):
                nc.tensor.matmul(
                    ps_w, warm, warm[:, :64], start=(i == 0), stop=(i == N_WARM - 1)
                )

            # ---- input loads. Emitted lazily (slot s+1 during phase s) so the
            # single DMA queue serves y writebacks promptly between input
            # groups instead of front-loading every input.
            w1_sb = [[None] * KF for _ in range(NQ)]
            w2_sb = [None] * NQ
            x_sb = [[None] * nchs[s] for s in range(NQ)]

            def load_w1(s, f):
                t = wpool.tile([P, KD, P], mybir.dt.bfloat16, name=f"w1_{s}_{f}")
                nc.sync.dma_start(t, w1_in[s][f])
                w1_sb[s][f] = t

            def load_x(s, c):
                t = apool.tile([P, KD, Ls[s]], mybir.dt.bfloat16, name=f"x_{s}_{c}")
                nc.sync.dma_start(t, x_in[s][c])
                x_sb[s][c] = t

            def load_w2(s):
                t = wpool.tile([P, KD, KF, P], mybir.dt.bfloat16, name=f"w2_{s}")
                nc.sync.dma_start(t, w2_in[s][:])
                w2_sb[s] = t

            # slots 0/1 + w2_0 up front, in consumption order
            load_w1(0, 0)
            load_x(0, 0)
            for f in range(1, KF):
                load_w1(0, f)
            for c in range(1, nchs[0]):
                load_x(0, c)
            load_x(1, 0)
            load_w1(1, 0)
            load_w2(0)
            for f in range(1, KF):
                load_w1(1, f)
            for c in range(1, nchs[1]):
                load_x(1, c)

            h_sb: dict = {}
            y_sb: dict = {}

            # stage 1 group: h.T[f-tile, chunk] = gelu(w1q[f] @ x.T[chunk])
            def g1(s, f, c):
                L = Ls[s]
                if (s, f) not in h_sb:
                    h_sb[(s, f)] = hpool.tile(
                        [P, Cmax],
                        mybir.dt.bfloat16,
                        name=f"h_{s}_{f}",
                        tag="h",
                        bufs=2 * KF,
                    )
                ps = ps1.tile([P, PSUM_FREE], mybir.dt.float32, name="ps1t", tag="ps1t")
                for k in range(KD):
                    nc.tensor.matmul(
                        ps[:, :L],
                        w1_sb[s][f][:, k],
                        x_sb[s][c][:, k],
                        start=(k == 0),
                        stop=(k == KD - 1),
                    )
                nc.scalar.activation(
                    out=h_sb[(s, f)][:, c * L : (c + 1) * L],
                    in_=ps[:, :L],
                    func=mybir.ActivationFunctionType.Gelu,
                )

            # stage 2 group: y.T[d-tile, chunk] = w2q[d] @ h.T[chunk] (partial)
            def g2(s, d, c):
                L = Ls[s]
                ps = ps2.tile([P, PSUM_FREE], mybir.dt.float32, name="ps2t", tag="ps2t")
                for k2 in range(KF):
                    nc.tensor.matmul(
                        ps[:, :L],
                        w2_sb[s][:, d, k2],
                        h_sb[(s, k2)][:, c * L : (c + 1) * L],
                        start=(k2 == 0),
                        stop=(k2 == KF - 1),
                    )
                if c == 0:
                    y_sb[(s, d)] = ypool.tile(
                        [P, Cmax], mybir.dt.bfloat16, name=f"y_{s}_{d}", tag="y", bufs=8
                    )
                nc.vector.tensor_copy(y_sb[(s, d)][:, c * L : (c + 1) * L], ps[:, :L])
                if c == nchs[s] - 1:
                    nc.sync.dma_start(y_out[s][d], y_sb[(s, d)][:, : Cs[s]])

            # ---- interleave stage-1 of slot s with stage-2 of slot s-1 so the
            # PE never stalls on the gelu eviction between stages. Chunk is
            # the OUTER loop on both sides: stage-2 chunk-c groups only need
            # h-chunks already produced a phase earlier.
            for phase in range(NQ + 1):
                if 1 <= phase <= NQ - 1:
                    # inputs for slot phase+1 and stage-2 weights for this phase
                    s2 = phase + 1
                    if s2 < NQ:
                        load_x(s2, 0)
                        for f in range(KF):
                            load_w1(s2, f)
                        for c in range(1, nchs[s2]):
                            load_x(s2, c)
                    load_w2(phase)
                g1s = (
                    [(phase, f, c) for c in range(nchs[phase]) for f in range(KF)]
                    if phase < NQ
                    else []
                )
                g2s = (
                    [
                        (phase - 1, d, c)
                        for c in range(nchs[phase - 1])
                        for d in range(KD)
                    ]
                    if phase >= 1
                    else []
                )
                for a, b in zip_longest(g1s, g2s):
                    if a is not None:
                        g1(*a)
                    if b is not None:
                        g2(*b)
    nc.finalize()
    return nc


def _route(x: np.ndarray, gate_w: np.ndarray):
    """fp64 gating; matches reference fp32 routing (selection gaps >> fp32 eps)."""
    logits = x.astype(np.float64) @ gate_w.astype(np.float64).T  # [N, E]
    top2 = np.argsort(-logits, axis=1, kind="stable")[:, :2]  # [N, 2]
    v = np.take_along_axis(logits, top2, axis=1)
    v = v - v.max(axis=1, keepdims=True)
    ew = np.exp(v)
    w = ew / ew.sum(axis=1, keepdims=True)  # [N, 2]
    return top2, w.astype(np.float32)


def _run(inputs: dict, trace: bool = False, trace_cores=None):
    x = np.asarray(inputs["x"], dtype=np.float32)
    gate_w = np.asarray(inputs["gate_w"], dtype=np.float32)
    w1 = np.asarray(inputs["w1"], dtype=np.float32)
    w2 = np.asarray(inputs["w2"], dtype=np.float32)
    n = x.shape[0]

    top2, wsm = _route(x, gate_w)

    idx_list, coef_list = [], []
    for e in range(E):
        mask = top2 == e  # [N, 2]
        sel = mask.any(axis=1)
        idx = np.nonzero(sel)[0]
        we = np.where(mask[idx, 0], wsm[idx, 0], wsm[idx, 1])
        idx_list.append(idx)
        coef_list.append(we.astype(np.float32))
    counts = np.array([len(i) for i in idx_list])

    # slot s holds the 2s-th and (2s+1)-th least-busy experts (ascending so
    # phase 0 -- the only phase with no stage-2 interleave -- is the shortest
    # and least DMA-pressured), snaked across the two quads to balance sums.
    order = np.argsort(counts, kind="stable")
    quad = [[0] * NQ for _ in range(2)]
    for s in range(NQ):
        a, b = int(order[2 * s]), int(order[2 * s + 1])
        if s % 2 == 1:
            a, b = b, a
        quad[0][s], quad[1][s] = a, b

    Cs, Ls, nchs = [], [], []
    for s in range(NQ):
        C, L, nch = _plan(int(max(counts[quad[0][s]], counts[quad[1][s]])))
        Cs.append(C)
        Ls.append(L)
        nchs.append(nch)

    key = (tuple(Cs), tuple(Ls))
    if key not in _NC_CACHE:
        _NC_CACHE[key] = _build_nc(Cs, Ls)
    nc = _NC_CACHE[key]

    in_maps = []
    for q in range(2):
        # packs shared by the 4 cores of a quad, except the F-slice
        xpacks = {}
        for s in range(NQ):
            e = quad[q][s]
            idx = idx_list[e]
            cnt = len(idx)
            C, L, nch = Cs[s], Ls[s], nchs[s]
            xT = np.zeros((D, C), dtype=BF16)
            xT[:, :cnt] = x[idx].T.astype(BF16)
            # [D, C] -> pack [c, p, k, t]
            xpacks[s] = np.ascontiguousarray(
                xT.reshape(KD, P, nch, L).transpose(2, 1, 0, 3)
            )
        for i in range(NQ):
            fsl = slice(i * FQ, (i + 1) * FQ)
            feed = {}
            for s in range(NQ):
                e = quad[q][s]
                feed[f"x_{s}"] = xpacks[s]
                # w1[e][fsl] is [FQ, D]: [f, c, k, p] -> pack [f, p, k, c]
                feed[f"w1_{s}"] = np.ascontiguousarray(
                    w1[e][fsl].astype(BF16).reshape(KF, P, KD, P).transpose(0, 3, 2, 1)
                )
                # w2[e][:, fsl] is [D, FQ]: [d, c, k2, p] -> pack [p, d, k2, c]
                feed[f"w2_{s}"] = np.ascontiguousarray(
                    w2[e][:, fsl]
                    .astype(BF16)
                    .reshape(KD, P, KF, P)
                    .transpose(3, 0, 2, 1)
                )
            in_maps.append(feed)

    res = run_bass_kernel_spmd(
        nc,
        in_maps,
        list(range(E)),
        trace=trace,
        trace_cores=trace_cores,
    )

    out = np.zeros((n, D), dtype=np.float32)
    for q in range(2):
        for s in range(NQ):
            e = quad[q][s]
            idx = idx_list[e]
            cnt = len(idx)
            ysum = np.zeros((D, Cs[s]), dtype=np.float32)
            for i in range(NQ):
                ysum += np.asarray(
                    res.results[4 * q + i][f"y_{s}"], dtype=np.float32
                ).reshape(D, Cs[s])
            out[idx] += (ysum[:, :cnt] * coef_list[e][None, :]).T
    return out, res


def kernel(**inputs) -> np.ndarray:
    out, _ = _run(inputs, trace=False)
    return out


if __name__ == "__main__":
    rng = np.random.default_rng(0)
    fake = {
        "x": rng.standard_normal((NTOK, D), dtype=np.float32),
        "gate_w": (rng.standard_normal((E, D)) * 0.02).astype(np.float32),
        "w1": (rng.standard_normal((E, F, D)) * 0.02).astype(np.float32),
        "w2": (rng.standard_normal((E, D, F)) * 0.02).astype(np.float32),
    }
    out = kernel(**fake)
    print("ok", out.shape, out.dtype, np.abs(out).max())


# revision 9
# speedup vs baseline: 1.0674x; 1.0098x over previous
"""MoE (top-2 routing, E=8 experts) Trainium2 kernel.

Strategy (quad expert-groups x F-sliced expert-parallel, 8 NeuronCores):
  - Host: gate in fp64, top-2 + softmax exactly reproducing reference routing.
  - Experts are ranked by routed-token count and paired into 4 "slots"
    (rank 2s with rank 2s+1); one expert of each slot goes to quad A
    (cores 0-3), the other to quad B (cores 4-7). Each core of a quad
    holds a distinct quarter of the F dimension (1024 rows of w1 /
    columns of w2) for all 4 of its quad's experts, and processes ALL
    routed tokens of those experts on its F-quarter. Per-core matmul
    work is therefore ~sum(slot capacities) * D * (F/4) * 2 regardless
    of routing imbalance -- near-perfect load balance (~110us of PE
    stream vs ~116us for 1-expert-per-core).
  - Device (per core, per slot): h.T = gelu(w1q @ x.T) [1024 x C],
    y.T += w2q @ h.T (partial over the F-quarter), evicted PSUM->SBUF
    as bf16 by the DVE and DMA'd out. Stage-1 of slot s+1 is interleaved
    with stage-2 of slot s at matmul-group granularity so the PE never
    waits on the gelu eviction.
  - Host: combine -- sum the 4 partial y's per quad (fp32), scale by the
    routing coefficient, scatter-add the two expert contributions.
"""

import sys
from itertools import zip_longest

import ml_dtypes
import numpy as np

if "/opt/trn_rl_repo" not in sys.path:
    sys.path.insert(0, "/opt/trn_rl_repo")

import concourse.tile as tile
from concourse import bacc, mybir
from concourse.bass_utils import run_bass_kernel_spmd

BF16 = ml_dtypes.bfloat16

E, D, F, NTOK = 8, 1024, 4096, 2048
P = 128
KD = D // P  # 8: stage-1 contraction tiles; also output d-tiles
NQ = 4  # experts per quad == F-quarters per quad
FQ = F // NQ  # 1024 rows of F per core
KF = FQ // P  # 8: f-tiles per slot; also stage-2 contraction tiles
PSUM_FREE = 512  # one PSUM bank of fp32

_NC_CACHE: dict = {}


def _plan(cnt_max: int):
    """Slot capacity C = nch * L, L <= 512, both multiples of 8."""
    C = max(16, -(-cnt_max // 8) * 8)
    nch = -(-C // 512)
    L = -(-C // (nch * 8)) * 8
    return nch * L, L, nch


def _build_nc(Cs, Ls):
    nchs = [c // l for c, l in zip(Cs, Ls)]
    Cmax = max(Cs)
    nc = bacc.Bacc(None)
    w1_in = [
        nc.declare_dram_parameter(
            f"w1_{s}", [KF, P, KD, P], mybir.dt.bfloat16, isOutput=False
        )
        for s in range(NQ)
    ]
    w2_in = [
        nc.declare_dram_parameter(
            f"w2_{s}", [P, KD, KF, P], mybir.dt.bfloat16, isOutput=False
        )
        for s in range(NQ)
    ]
    x_in = [
        nc.declare_dram_parameter(
            f"x_{s}", [nchs[s], P, KD, Ls[s]], mybir.dt.bfloat16, isOutput=False
        )
        for s in range(NQ)
    ]
    y_out = [
        nc.declare_dram_parameter(
            f"y_{s}", [KD, P, Cs[s]], mybir.dt.bfloat16, isOutput=True
        )
        for s in range(NQ)
    ]

    with tile.TileContext(nc) as tc:
        with (
            tc.tile_pool(name="wpool", bufs=1) as wpool,
            tc.tile_pool(name="apool", bufs=1) as apool,
            tc.tile_pool(name="hpool", bufs=1) as hpool,
            tc.tile_pool(name="ypool", bufs=1) as ypool,
            tc.tile_pool(name="ps1", bufs=4, space="PSUM") as ps1,
            tc.tile_pool(name="ps2", bufs=3, space="PSUM") as ps2,
            tc.tile_pool(name="psw", bufs=1, space="PSUM") as psw,
        ):
            # ---- PE warm-up: dependency-free matmuls during the DMA head so
            # the HAM clock gate is at 8/8 by the time the real stream starts
            # (the first x/w1 DMAs land ~7us after the PE frees up).
            warm = wpool.tile([P, P], mybir.dt.bfloat16, name="warm")
            nc.vector.memset(warm, 0.0)
            # 128-free matmuls saturate the PE at cold clock (107ns busy vs
            # ~53ns issue), so the HAM sees a fully-busy window and
            # un-throttles mid-warm-up; 52 of them span ~5.6us cold.
            N_WARM = 52
            ps_w = psw.tile([P, P], mybir.dt.float32, name="ps_w")
            for i in range(N_WARM):
                nc.tensor.matmul(
                    ps_w, warm, warm, start=(i == 0), stop=(i == N_WARM - 1)
                )

            # ---- input loads. Emitted lazily (slot s+1 during phase s) so the
            # single DMA queue serves y writebacks promptly between input
            # groups instead of front-loading every input.
            w1_sb = [[None] * KF for _ in range(NQ)]
            w2_sb = [None] * NQ
            x_sb = [[None] * nchs[s] for s in range(NQ)]

            def load_w1(s, f):
                t = wpool.tile([P, KD, P], mybir.dt.bfloat16, name=f"w1_{s}_{f}")
                nc.sync.dma_start(t, w1_in[s][f])
                w1_sb[s][f] = t

            def load_x(s, c):
                t = apool.tile([P, KD, Ls[s]], mybir.dt.bfloat16, name=f"x_{s}_{c}")
                nc.sync.dma_start(t, x_in[s][c])
                x_sb[s][c] = t

            def load_w2(s):
                t = wpool.tile([P, KD, KF, P], mybir.dt.bfloat16, name=f"w2_{s}")
                nc.sync.dma_start(t, w2_in[s][:])
                w2_sb[s] = t

            # slots 0/1 + w2_0 up front, in consumption order
            load_w1(0, 0)
            load_x(0, 0)
            for f in range(1, KF):
                load_w1(0, f)
            for c in range(1, nchs[0]):
                load_x(0, c)
            load_x(1, 0)
            load_w1(1, 0)
            load_w2(0)
            for f in range(1, KF):
                load_w1(1, f)
            for c in range(1, nchs[1]):
                load_x(1, c)

            h_sb: dict = {}
            y_sb: dict = {}

            # stage 1 group: h.T[f-tile, chunk] = gelu(w1q[f] @ x.T[chunk])
            def g1(s, f, c):
                L = Ls[s]
                if (s, f) not in h_sb:
                    h_sb[(s, f)] = hpool.tile(
                        [P, Cmax],
                        mybir.dt.bfloat16,
                        name=f"h_{s}_{f}",
                        tag="h",
                        bufs=2 * KF,
                    )
                ps = ps1.tile([P, PSUM_FREE], mybir.dt.float32, name="ps1t", tag="ps1t")
                for k in range(KD):
                    nc.tensor.matmul(
                        ps[:, :L],
                        w1_sb[s][f][:, k],
                        x_sb[s][c][:, k],
                        start=(k == 0),
                        stop=(k == KD - 1),
                    )
                nc.scalar.activation(
                    out=h_sb[(s, f)][:, c * L : (c + 1) * L],
                    in_=ps[:, :L],
                    func=mybir.ActivationFunctionType.Gelu,
                )

            # stage 2 group: y.T[d-tile, chunk] = w2q[d] @ h.T[chunk] (partial)
            def g2(s, d, c):
                L = Ls[s]
                ps = ps2.tile([P, PSUM_FREE], mybir.dt.float32, name="ps2t", tag="ps2t")
                for k2 in range(KF):
                    nc.tensor.matmul(
                        ps[:, :L],
                        w2_sb[s][:, d, k2],
                        h_sb[(s, k2)][:, c * L : (c + 1) * L],
                        start=(k2 == 0),
                        stop=(k2 == KF - 1),
                    )
                if c == 0:
                    y_sb[(s, d)] = ypool.tile(
                        [P, Cmax], mybir.dt.bfloat16, name=f"y_{s}_{d}", tag="y", bufs=8
                    )
                nc.vector.tensor_copy(y_sb[(s, d)][:, c * L : (c + 1) * L], ps[:, :L])
                if c == nchs[s] - 1:
                    # gpsimd's DMA queue: y writebacks never queue behind inputs
                    nc.gpsimd.dma_start(y_out[s][d], y_sb[(s, d)][:, : Cs[s]])

            # ---- interleave stage-1 of slot s with stage-2 of slot s-1 so the
            # PE never stalls on the gelu eviction between stages. Chunk is
            # the OUTER loop on both sides: stage-2 chunk-c groups only need
            # h-chunks already produced a phase earlier.
            for phase in range(NQ + 1):
                if 1 <= phase <= NQ - 1:
                    # inputs for slot phase+1 and stage-2 weights for this phase
                    s2 = phase + 1
                    if s2 < NQ:
                        load_x(s2, 0)
                        for f in range(KF):
                            load_w1(s2, f)
                        for c in range(1, nchs[s2]):
                            load_x(s2, c)
                    load_w2(phase)
                g1s = (
                    [(phase, f, c) for c in range(nchs[phase]) for f in range(KF)]
                    if phase < NQ
                    else []
                )
                g2s = (
                    [
                        (phase - 1, d, c)
                        for c in range(nchs[phase - 1])
                        for d in range(KD)
                    ]
                    if phase >= 1
                    else []
                )
                for a, b in zip_longest(g1s, g2s):
                    if a is not None:
                        g1(*a)
                    if b is not None:
                        g2(*b)
    nc.finalize()
    return nc


def _route(x: np.ndarray, gate_w: np.ndarray):
    """fp64 gating; matches reference fp32 routing (selection gaps >> fp32 eps)."""
    logits = x.astype(np.float64) @ gate_w.astype(np.float64).T  # [N, E]
    top2 = np.argsort(-logits, axis=1, kind="stable")[:, :2]  # [N, 2]
    v = np.take_along_axis(logits, top2, axis=1)
    v = v - v.max(axis=1, keepdims=True)
    ew = np.exp(v)
    w = ew / ew.sum(axis=1, keepdims=True)  # [N, 2]
    return top2, w.astype(np.float32)


def _run(inputs: dict, trace: bool = False, trace_cores=None):
    x = np.asarray(inputs["x"], dtype=np.float32)
    gate_w = np.asarray(inputs["gate_w"], dtype=np.float32)
    w1 = np.asarray(inputs["w1"], dtype=np.float32)
    w2 = np.asarray(inputs["w2"], dtype=np.float32)
    n = x.shape[0]

    top2, wsm = _route(x, gate_w)

    idx_list, coef_list = [], []
    for e in range(E):
        mask = top2 == e  # [N, 2]
        sel = mask.any(axis=1)
        idx = np.nonzero(sel)[0]
        we = np.where(mask[idx, 0], wsm[idx, 0], wsm[idx, 1])
        idx_list.append(idx)
        coef_list.append(we.astype(np.float32))
    counts = np.array([len(i) for i in idx_list])

    # slot s holds the 2s-th and (2s+1)-th least-busy experts (ascending so
    # phase 0 -- the only phase with no stage-2 interleave -- is the shortest
    # and least DMA-pressured), snaked across the two quads to balance sums.
    order = np.argsort(counts, kind="stable")
    quad = [[0] * NQ for _ in range(2)]
    for s in range(NQ):
        a, b = int(order[2 * s]), int(order[2 * s + 1])
        if s % 2 == 1:
            a, b = b, a
        quad[0][s], quad[1][s] = a, b

    Cs, Ls, nchs = [], [], []
    for s in range(NQ):
        C, L, nch = _plan(int(max(counts[quad[0][s]], counts[quad[1][s]])))
        Cs.append(C)
        Ls.append(L)
        nchs.append(nch)

    key = (tuple(Cs), tuple(Ls))
    if key not in _NC_CACHE:
        _NC_CACHE[key] = _build_nc(Cs, Ls)
    nc = _NC_CACHE[key]

    in_maps = []
    for q in range(2):
        # packs shared by the 4 cores of a quad, except the F-slice
        xpacks = {}
        for s in range(NQ):
            e = quad[q][s]
            idx = idx_list[e]
            cnt = len(idx)
            C, L, nch = Cs[s], Ls[s], nchs[s]
            xT = np.zeros((D, C), dtype=BF16)
            xT[:, :cnt] = x[idx].T.astype(BF16)
            # [D, C] -> pack [c, p, k, t]
            xpacks[s] = np.ascontiguousarray(
                xT.reshape(KD, P, nch, L).transpose(2, 1, 0, 3)
            )
        for i in range(NQ):
            fsl = slice(i * FQ, (i + 1) * FQ)
            feed = {}
            for s in range(NQ):
                e = quad[q][s]
                feed[f"x_{s}"] = xpacks[s]
                # w1[e][fsl] is [FQ, D]: [f, c, k, p] -> pack [f, p, k, c]
                feed[f"w1_{s}"] = np.ascontiguousarray(
                    w1[e][fsl].astype(BF16).reshape(KF, P, KD, P).transpose(0, 3, 2, 1)
                )
                # w2[e][:, fsl] is [D, FQ]: [d, c, k2, p] -> pack [p, d, k2, c]
                feed[f"w2_{s}"] = np.ascontiguousarray(
                    w2[e][:, fsl]
                    .astype(BF16)
                    .reshape(KD, P, KF, P)
                    .transpose(3, 0, 2, 1)
                )
            in_maps.append(feed)

    res = run_bass_kernel_spmd(
        nc,
        in_maps,
        list(range(E)),
        trace=trace,
        trace_cores=trace_cores,
    )

    out = np.zeros((n, D), dtype=np.float32)
    for q in range(2):
        for s in range(NQ):
            e = quad[q][s]
            idx = idx_list[e]
            cnt = len(idx)
            ysum = np.zeros((D, Cs[s]), dtype=np.float32)
            for i in range(NQ):
                ysum += np.asarray(
                    res.results[4 * q + i][f"y_{s}"], dtype=np.float32
                ).reshape(D, Cs[s])
            out[idx] += (ysum[:, :cnt] * coef_list[e][None, :]).T
    return out, res


def kernel(**inputs) -> np.ndarray:
    out, _ = _run(inputs, trace=False)
    return out


if __name__ == "__main__":
    rng = np.random.default_rng(0)
    fake = {
        "x": rng.standard_normal((NTOK, D), dtype=np.float32),
        "gate_w": (rng.standard_normal((E, D)) * 0.02).astype(np.float32),
        "w1": (rng.standard_normal((E, F, D)) * 0.02).astype(np.float32),
        "w2": (rng.standard_normal((E, D, F)) * 0.02).astype(np.float32),
    }
    out = kernel(**fake)
    print("ok", out.shape, out.dtype, np.abs(out).max())


# revision 13
# speedup vs baseline: 1.0709x; 1.0033x over previous
"""MoE (top-2 routing, E=8 experts) Trainium2 kernel.

Strategy (quad expert-groups x F-sliced expert-parallel, 8 NeuronCores):
  - Host: gate in fp64, top-2 + softmax exactly reproducing reference routing.
  - Experts are ranked by routed-token count and paired into 4 "slots"
    (rank 2s with rank 2s+1); one expert of each slot goes to quad A
    (cores 0-3), the other to quad B (cores 4-7). Each core of a quad
    holds a distinct quarter of the F dimension (1024 rows of w1 /
    columns of w2) for all 4 of its quad's experts, and processes ALL
    routed tokens of those experts on its F-quarter. Per-core matmul
    work is therefore ~sum(slot capacities) * D * (F/4) * 2 regardless
    of routing imbalance -- near-perfect load balance (~110us of PE
    stream vs ~116us for 1-expert-per-core).
  - Device (per core, per slot): h.T = gelu(w1q @ x.T) [1024 x C],
    y.T += w2q @ h.T (partial over the F-quarter), evicted PSUM->SBUF
    as bf16 by the DVE and DMA'd out. Stage-1 of slot s+1 is interleaved
    with stage-2 of slot s at matmul-group granularity so the PE never
    waits on the gelu eviction.
  - Host: combine -- sum the 4 partial y's per quad (fp32), scale by the
    routing coefficient, scatter-add the two expert contributions.
"""

import sys
from itertools import zip_longest

import ml_dtypes
import numpy as np

if "/opt/trn_rl_repo" not in sys.path:
    sys.path.insert(0, "/opt/trn_rl_repo")

import concourse.tile as tile
from concourse import bacc, mybir
from concourse.bass_utils import run_bass_kernel_spmd

BF16 = ml_dtypes.bfloat16

E, D, F, NTOK = 8, 1024, 4096, 2048
P = 128
KD = D // P  # 8: stage-1 contraction tiles; also output d-tiles
NQ = 4  # experts per quad == F-quarters per quad
FQ = F // NQ  # 1024 rows of F per core
KF = FQ // P  # 8: f-tiles per slot; also stage-2 contraction tiles
PSUM_FREE = 512  # one PSUM bank of fp32

_NC_CACHE: dict = {}


def _plan(cnt_max: int):
    """Slot capacity C = nch * L, L <= 512, both multiples of 8."""
    C = max(16, -(-cnt_max // 8) * 8)
    nch = -(-C // 512)
    L = -(-C // (nch * 8)) * 8
    return nch * L, L, nch


def _build_nc(Cs, Ls):
    nchs = [c // l for c, l in zip(Cs, Ls)]
    Cmax = max(Cs)
    nc = bacc.Bacc(None)
    w1_in = [
        nc.declare_dram_parameter(
            f"w1_{s}", [KF, P, KD, P], mybir.dt.bfloat16, isOutput=False
        )
        for s in range(NQ)
    ]
    w2_in = [
        nc.declare_dram_parameter(
            f"w2_{s}", [P, KD, KF, P], mybir.dt.bfloat16, isOutput=False
        )
        for s in range(NQ)
    ]
    x_in = [
        nc.declare_dram_parameter(
            f"x_{s}", [nchs[s], P, KD, Ls[s]], mybir.dt.bfloat16, isOutput=False
        )
        for s in range(NQ)
    ]
    y_out = [
        nc.declare_dram_parameter(
            f"y_{s}", [KD, P, Cs[s]], mybir.dt.bfloat16, isOutput=True
        )
        for s in range(NQ)
    ]

    with tile.TileContext(nc) as tc:
        with (
            tc.tile_pool(name="wpool", bufs=1) as wpool,
            tc.tile_pool(name="apool", bufs=1) as apool,
            tc.tile_pool(name="hpool", bufs=1) as hpool,
            tc.tile_pool(name="ypool", bufs=1) as ypool,
            tc.tile_pool(name="ps1", bufs=4, space="PSUM") as ps1,
            tc.tile_pool(name="ps2", bufs=3, space="PSUM") as ps2,
            tc.tile_pool(name="psw", bufs=1, space="PSUM") as psw,
        ):
            # ---- PE warm-up: dependency-free matmuls during the DMA head so
            # the HAM clock gate is at 8/8 by the time the real stream starts
            # (the first x/w1 DMAs land ~7us after the PE frees up).
            warm = wpool.tile([P, P], mybir.dt.bfloat16, name="warm")
            nc.vector.memset(warm, 0.0)
            # 128-free matmuls saturate the PE at cold clock (107ns busy vs
            # ~53ns issue), so the HAM sees a fully-busy window and
            # un-throttles mid-warm-up; 52 of them span ~5.6us cold.
            N_WARM = 52
            ps_w = psw.tile([P, P], mybir.dt.float32, name="ps_w")
            for i in range(N_WARM):
                nc.tensor.matmul(
                    ps_w, warm, warm, start=(i == 0), stop=(i == N_WARM - 1)
                )

            # ---- input loads. Emitted lazily (slot s+1 during phase s) so the
            # single DMA queue serves y writebacks promptly between input
            # groups instead of front-loading every input.
            w1_sb = [[None] * KF for _ in range(NQ)]
            w2_sb = [None] * NQ
            x_sb = [[None] * nchs[s] for s in range(NQ)]

            def load_w1(s, f):
                t = wpool.tile([P, KD, P], mybir.dt.bfloat16, name=f"w1_{s}_{f}")
                nc.sync.dma_start(t, w1_in[s][f])
                w1_sb[s][f] = t

            def load_x(s, c, engine=None):
                t = apool.tile([P, KD, Ls[s]], mybir.dt.bfloat16, name=f"x_{s}_{c}")
                (engine or nc.sync).dma_start(t, x_in[s][c])
                x_sb[s][c] = t

            def load_w2(s):
                t = wpool.tile([P, KD, KF, P], mybir.dt.bfloat16, name=f"w2_{s}")
                nc.sync.dma_start(t, w2_in[s][:])
                w2_sb[s] = t

            # slots 0/1 + w2_0 up front, in consumption order. x_0 goes on the
            # gpsimd engine's DMA queue, in parallel with w1_0 on sync's --
            # both gate the first real matmul.
            load_w1(0, 0)
            load_x(0, 0, engine=nc.gpsimd)
            for f in range(1, KF):
                load_w1(0, f)
            for c in range(1, nchs[0]):
                load_x(0, c)
            load_x(1, 0)
            load_w1(1, 0)
            load_w2(0)
            for f in range(1, KF):
                load_w1(1, f)
            for c in range(1, nchs[1]):
                load_x(1, c)

            h_sb: dict = {}
            y_sb: dict = {}

            # stage 1 group: h.T[f-tile, chunk] = gelu(w1q[f] @ x.T[chunk])
            def g1(s, f, c):
                L = Ls[s]
                if (s, f) not in h_sb:
                    h_sb[(s, f)] = hpool.tile(
                        [P, Cmax],
                        mybir.dt.bfloat16,
                        name=f"h_{s}_{f}",
                        tag="h",
                        bufs=2 * KF,
                    )
                ps = ps1.tile([P, PSUM_FREE], mybir.dt.float32, name="ps1t", tag="ps1t")
                for k in range(KD):
                    nc.tensor.matmul(
                        ps[:, :L],
                        w1_sb[s][f][:, k],
                        x_sb[s][c][:, k],
                        start=(k == 0),
                        stop=(k == KD - 1),
                    )
                nc.scalar.activation(
                    out=h_sb[(s, f)][:, c * L : (c + 1) * L],
                    in_=ps[:, :L],
                    func=mybir.ActivationFunctionType.Gelu,
                )

            # stage 2 group: y.T[d-tile, chunk] = w2q[d] @ h.T[chunk] (partial)
            def g2(s, d, c):
                L = Ls[s]
                ps = ps2.tile([P, PSUM_FREE], mybir.dt.float32, name="ps2t", tag="ps2t")
                for k2 in range(KF):
                    nc.tensor.matmul(
                        ps[:, :L],
                        w2_sb[s][:, d, k2],
                        h_sb[(s, k2)][:, c * L : (c + 1) * L],
                        start=(k2 == 0),
                        stop=(k2 == KF - 1),
                    )
                if c == 0:
                    y_sb[(s, d)] = ypool.tile(
                        [P, Cmax], mybir.dt.bfloat16, name=f"y_{s}_{d}", tag="y", bufs=8
                    )
                nc.vector.tensor_copy(y_sb[(s, d)][:, c * L : (c + 1) * L], ps[:, :L])
                # per-chunk writeback so the final drain only waits on the
                # last chunk, not the whole slot
                nc.sync.dma_start(
                    y_out[s][d][:, c * L : (c + 1) * L],
                    y_sb[(s, d)][:, c * L : (c + 1) * L],
                )

            # ---- interleave stage-1 of slot s with stage-2 of slot s-1 so the
            # PE never stalls on the gelu eviction between stages. Chunk is
            # the OUTER loop on both sides: stage-2 chunk-c groups only need
            # h-chunks already produced a phase earlier.
            for phase in range(NQ + 1):
                if 1 <= phase <= NQ - 1:
                    # inputs for slot phase+1 and stage-2 weights for this phase
                    s2 = phase + 1
                    if s2 < NQ:
                        load_x(s2, 0)
                        for f in range(KF):
                            load_w1(s2, f)
                        for c in range(1, nchs[s2]):
                            load_x(s2, c)
                    load_w2(phase)
                g1s = (
                    [(phase, f, c) for c in range(nchs[phase]) for f in range(KF)]
                    if phase < NQ
                    else []
                )
                g2s = (
                    [
                        (phase - 1, d, c)
                        for c in range(nchs[phase - 1])
                        for d in range(KD)
                    ]
                    if phase >= 1
                    else []
                )
                for a, b in zip_longest(g1s, g2s):
                    if a is not None:
                        g1(*a)
                    if b is not None:
                        g2(*b)
    nc.finalize()
    return nc


def _route(x: np.ndarray, gate_w: np.ndarray):
    """fp64 gating; matches reference fp32 routing (selection gaps >> fp32 eps)."""
    logits = x.astype(np.float64) @ gate_w.astype(np.float64).T  # [N, E]
    top2 = np.argsort(-logits, axis=1, kind="stable")[:, :2]  # [N, 2]
    v = np.take_along_axis(logits, top2, axis=1)
    v = v - v.max(axis=1, keepdims=True)
    ew = np.exp(v)
    w = ew / ew.sum(axis=1, keepdims=True)  # [N, 2]
    return top2, w.astype(np.float32)


def _run(inputs: dict, trace: bool = False, trace_cores=None):
    x = np.asarray(inputs["x"], dtype=np.float32)
    gate_w = np.asarray(inputs["gate_w"], dtype=np.float32)
    w1 = np.asarray(inputs["w1"], dtype=np.float32)
    w2 = np.asarray(inputs["w2"], dtype=np.float32)
    n = x.shape[0]

    top2, wsm = _route(x, gate_w)

    idx_list, coef_list = [], []
    for e in range(E):
        mask = top2 == e  # [N, 2]
        sel = mask.any(axis=1)
        idx = np.nonzero(sel)[0]
        we = np.where(mask[idx, 0], wsm[idx, 0], wsm[idx, 1])
        idx_list.append(idx)
        coef_list.append(we.astype(np.float32))
    counts = np.array([len(i) for i in idx_list])

    # slot s holds the 2s-th and (2s+1)-th least-busy experts (ascending so
    # phase 0 -- the only phase with no stage-2 interleave -- is the shortest
    # and least DMA-pressured), snaked across the two quads to balance sums.
    order = np.argsort(counts, kind="stable")
    quad = [[0] * NQ for _ in range(2)]
    for s in range(NQ):
        a, b = int(order[2 * s]), int(order[2 * s + 1])
        if s % 2 == 1:
            a, b = b, a
        quad[0][s], quad[1][s] = a, b

    Cs, Ls, nchs = [], [], []
    for s in range(NQ):
        C, L, nch = _plan(int(max(counts[quad[0][s]], counts[quad[1][s]])))
        Cs.append(C)
        Ls.append(L)
        nchs.append(nch)

    key = (tuple(Cs), tuple(Ls))
    if key not in _NC_CACHE:
        _NC_CACHE[key] = _build_nc(Cs, Ls)
    nc = _NC_CACHE[key]

    in_maps = []
    for q in range(2):
        # packs shared by the 4 cores of a quad, except the F-slice
        xpacks = {}
        for s in range(NQ):
            e = quad[q][s]
            idx = idx_list[e]
            cnt = len(idx)
            C, L, nch = Cs[s], Ls[s], nchs[s]
            xT = np.zeros((D, C), dtype=BF16)
            xT[:, :cnt] = x[idx].T.astype(BF16)
            # [D, C] -> pack [c, p, k, t]
            xpacks[s] = np.ascontiguousarray(
                xT.reshape(KD, P, nch, L).transpose(2, 1, 0, 3)
            )
        for i in range(NQ):
            fsl = slice(i * FQ, (i + 1) * FQ)
            feed = {}
            for s in range(NQ):
                e = quad[q][s]
                feed[f"x_{s}"] = xpacks[s]
                # w1[e][fsl] is [FQ, D]: [f, c, k, p] -> pack [f, p, k, c]
                feed[f"w1_{s}"] = np.ascontiguousarray(
                    w1[e][fsl].astype(BF16).reshape(KF, P, KD, P).transpose(0, 3, 2, 1)
                )
                # w2[e][:, fsl] is [D, FQ]: [d, c, k2, p] -> pack [p, d, k2, c]
                feed[f"w2_{s}"] = np.ascontiguousarray(
                    w2[e][:, fsl]
                    .astype(BF16)
                    .reshape(KD, P, KF, P)
                    .transpose(3, 0, 2, 1)
                )
            in_maps.append(feed)

    res = run_bass_kernel_spmd(
        nc,
        in_maps,
        list(range(E)),
        trace=trace,
        trace_cores=trace_cores,
    )

    out = np.zeros((n, D), dtype=np.float32)
    for q in range(2):
        for s in range(NQ):
            e = quad[q][s]
            idx = idx_list[e]
            cnt = len(idx)
            ysum = np.zeros((D, Cs[s]), dtype=np.float32)
            for i in range(NQ):
                ysum += np.asarray(
                    res.results[4 * q + i][f"y_{s}"], dtype=np.float32
                ).reshape(D, Cs[s])
            out[idx] += (ysum[:, :cnt] * coef_list[e][None, :]).T
    return out, res


def kernel(**inputs) -> np.ndarray:
    out, _ = _run(inputs, trace=False)
    return out


if __name__ == "__main__":
    rng = np.random.default_rng(0)
    fake = {
        "x": rng.standard_normal((NTOK, D), dtype=np.float32),
        "gate_w": (rng.standard_normal((E, D)) * 0.02).astype(np.float32),
        "w1": (rng.standard_normal((E, F, D)) * 0.02).astype(np.float32),
        "w2": (rng.standard_normal((E, D, F)) * 0.02).astype(np.float32),
    }
    out = kernel(**fake)
    print("ok", out.shape, out.dtype, np.abs(out).max())
